# revision 36
# baseline (speedup 1.0000x reference)
"""Multi-head attention block (qkv proj -> softmax attention -> out proj)
for B=2, N=2048, C=1024, H=16 heads of d=64, distributed over 8 NeuronCores.

Sharding: core c = (b, g) with b = c // 4 (batch), g = c % 4 (head group of
4 heads). Each core computes q/k/v for its 4 heads, full softmax attention,
and a partial output projection (its 256 input channels of w_proj). The
host sums the 4 per-batch partials (bf16) and adds b_proj.

The kernel is paced by the ACT engine: softmax exp of 16.8M elements/core
runs at ~1.2us per [128,1024] tile (the hard floor; exp exists only on
ACT). Everything else is scheduled to hide under that pace:
  - PSUM is partitioned into dedicated pools (scores 2x2 banks, PV
    accumulators 2x1, filler matmuls 2x1) so the score pipeline never
    chains through qkv/v/proj pool-slot reuse.
  - qkv / v / proj matmuls are emitted at low priority in consumption
    order, interleaved into the 128-step score->exp->PV loop, instead of
    as a serial prologue/epilogue.
  - proj for query-chunk ic runs as soon as both head-pairs' rounds for
    ic are normalized (round order is hp-outer), leaving only the last
    chunk's projection in the tail.
  - y partials are written bf16 (host accumulates in fp32).

Device layout notes (per core):
  - xT [1024, 2048] = x[b].T so the contraction dim (C) lands on SBUF
    partitions for both qkv orientations.
  - q/k are produced transposed ([head_dim, tokens]); consecutive heads sit
    at partition offsets 0 / 64 so the two K=64 score matmuls of a head
    pair occupy disjoint PE row groups and run concurrently (row tiling).
  - v is produced in [tokens, head_dim] layout with an extra all-ones
    column per head; the PV matmul then yields both the unnormalized
    attention output and the softmax denominator Z in one pass.
  - softmax has no max-subtraction: scores are ~N(0,1) (|S*scale| < ~8),
    safely inside fp32 exp range. SCALE is folded into the host-side q
    weights so exp needs no scale argument.
"""

import sys
import types

import numpy as np
import ml_dtypes

B = 2
N = 2048
C = 1024
H = 16
D = 64
HL = H // 4          # heads per core = 4
SCALE = D ** -0.5
N_CORES = 8
KT = C // 128        # 8 contraction tiles
MT = N // 128        # 16 token tiles
BF = ml_dtypes.bfloat16

_cache = {}


def _install_ntff_hook():
    """Register the axon NTFF profiling hook that this image's antenv lacks
    (profiling degrades gracefully without it; needed for exec_time_ns)."""
    try:
        import antenv.axon_hooks  # noqa: F401
        return
    except ImportError:
        pass
    try:
        import antenv
        from trn_agent_boot.trn_boot import _ntff_profile_via_ctypes
    except ImportError:
        return
    mod = types.ModuleType("antenv.axon_hooks")
    _hook = [None]
    mod.set_axon_ntff_profile_hook = lambda h: _hook.__setitem__(0, h)
    mod.get_axon_ntff_profile_hook = lambda: _hook[0]
    sys.modules["antenv.axon_hooks"] = mod
    antenv.axon_hooks = mod
    try:
        mod.set_axon_ntff_profile_hook(
            _ntff_profile_via_ctypes("/opt/axon/libaxon_pjrt.so")
        )
    except Exception:
        pass


def _build_program(v_bias_nonzero: bool, debug: bool = False):
    from contextlib import ExitStack

    import concourse.bass as bass
    import concourse.tile as tile
    from concourse import bacc, mybir

    f32 = mybir.dt.float32
    bf16 = mybir.dt.bfloat16
    Exp = mybir.ActivationFunctionType.Exp
    add = mybir.AluOpType.add

    nc = bacc.Bacc("TRN2", target_bir_lowering=False, debug=False,
                   num_devices=N_CORES)

    xT_d = nc.dram_tensor("xT", [C, N], bf16, kind="ExternalInput").ap()
    wqk_d = nc.dram_tensor("wqk", [C, 512], bf16, kind="ExternalInput").ap()
    wv_d = nc.dram_tensor("wv", [C, 256], bf16, kind="ExternalInput").ap()
    wp_d = nc.dram_tensor("wp", [256, C], bf16, kind="ExternalInput").ap()
    bqk_d = nc.dram_tensor("bqk", [512, 1], f32, kind="ExternalInput").ap()
    bv_d = nc.dram_tensor("bv", [64, 4], f32, kind="ExternalInput").ap()
    y_d = nc.dram_tensor("y", [N, C], bf16, kind="ExternalOutput").ap()
    warm_d = nc.dram_tensor("warm", [1, 8], f32, kind="ExternalOutput").ap()

    with tile.TileContext(nc) as tc, ExitStack() as ctx:
        persist = ctx.enter_context(tc.tile_pool(name="persist", bufs=1))
        # PSUM budget (8 banks): scores 2x[128,1024] (4 banks, double
        # buffered against the exp reader), pv 2x[128,512] (the long-lived
        # PV accumulators), mm 2x[128,512] (qkv / v / proj filler rotation).
        s_pool = ctx.enter_context(
            tc.tile_pool(name="s", bufs=2, space="PSUM"))
        pv_pool = ctx.enter_context(
            tc.tile_pool(name="pv", bufs=2, space="PSUM"))
        mm_pool = ctx.enter_context(
            tc.tile_pool(name="mm", bufs=2, space="PSUM"))
        es_pool = ctx.enter_context(tc.tile_pool(name="es", bufs=24))
        z_pool = ctx.enter_context(tc.tile_pool(name="z", bufs=3))
        y_pool = ctx.enter_context(tc.tile_pool(name="ysb", bufs=4))
        zd_pool = ctx.enter_context(
            tc.tile_pool(name="zd", bufs=4, space="DRAM"))

        xT = persist.tile([128, KT, N], bf16)
        wqk = persist.tile([128, KT, 512], bf16)
        wv = persist.tile([128, KT, 256], bf16)
        wp = persist.tile([128, 2, C], bf16)
        bq = persist.tile([128, 4], f32)
        bv = persist.tile([64, 4], f32) if v_bias_nonzero else None
        # q/k activations split into per-(dim-tile, token-chunk) tiles so the
        # scheduler releases attention matmuls as soon as each chunk lands
        qkT = [[persist.tile([128, 512], bf16, name=f"qkT{nt}_{mc}")
                for mc in range(4)] for nt in range(4)]
        v_sb = persist.tile([128, MT, HL * 65], bf16)
        out_sb = persist.tile([128, 2, N], bf16)
        warm_sb = persist.tile([1, 8], f32)

        # xT on the sync+scalar queues (one queue can nearly saturate the
        # shared DMA engine pool; two keeps descriptor issue off the
        # critical path), weights on gpsimd ordered by first use: k then q
        # of head pair 0, v, then pair 1, then the projection weights.
        for kt in range(KT):
            q = nc.sync if kt % 2 == 0 else nc.scalar
            q.dma_start(xT[:, kt, :], xT_d[kt * 128:(kt + 1) * 128, :])
        def wqk_slices(nt):
            for kt in range(KT):
                nc.gpsimd.dma_start(
                    wqk[:, kt, nt * 128:(nt + 1) * 128],
                    wqk_d[kt * 128:(kt + 1) * 128, nt * 128:(nt + 1) * 128])
        wqk_slices(2)
        wqk_slices(0)
        for kt in range(KT):
            nc.gpsimd.dma_start(wv[:, kt, :], wv_d[kt * 128:(kt + 1) * 128, :])
        for nt in (3, 1):
            wqk_slices(nt)
        for ct in range(2):
            nc.gpsimd.dma_start(wp[:, ct, :], wp_d[ct * 128:(ct + 1) * 128, :])
        # bqk[512,1] -> [128 partitions, 4 tiles]
        nc.sync.dma_start(bq[:], bqk_d.rearrange("(t p) o -> p (t o)", p=128))
        if v_bias_nonzero:
            # bv[64, 4]: column h = bias of head h, partitions 0-63
            nc.sync.dma_start(bv[:], bv_d[:])

        # warm-up exp (after the scalar-queue weight DMAs): pulls the ACT
        # table load off the critical path
        nc.vector.memset(warm_sb[:], 0.0)
        nc.scalar.activation(warm_sb[:], warm_sb[:], Exp)
        nc.sync.dma_start(warm_d[:], warm_sb[:])



        def qk_block(nt, mc):
            ps = mm_pool.tile([128, 512], f32, tag="mm", name=f"qk{nt}_{mc}")
            for kt in range(KT):
                nc.tensor.matmul(
                    ps[:],
                    lhsT=wqk[:, kt, nt * 128:(nt + 1) * 128],
                    rhs=xT[:, kt, mc * 512:(mc + 1) * 512],
                    start=(kt == 0), stop=(kt == KT - 1))
            nc.vector.tensor_scalar(
                out=qkT[nt][mc][:], in0=ps[:],
                scalar1=bq[:, nt:nt + 1], scalar2=None, op0=add)

        def v_block(mt):
            ps = mm_pool.tile([128, 512], f32, tag="mm", name=f"v{mt}")
            for kt in range(KT):
                nc.tensor.matmul(
                    ps[:, 0:256],
                    lhsT=xT[:, kt, mt * 128:(mt + 1) * 128],
                    rhs=wv[:, kt, :],
                    start=(kt == 0), stop=(kt == KT - 1))
            # v_aug per head = [v | ones]: the PV matmul then puts v at
            # psum partitions 0..63 and the denominator Z at partition 64
            dst = v_sb[:, mt, :].rearrange("p (h c) -> p h c", c=65)
            nc.vector.tensor_copy(
                dst[:, :, 0:64],
                ps[:, 0:256].rearrange("p (h c) -> p h c", c=64))
            nc.vector.memset(dst[:, :, 64:65], 1.0)

        # One step = one j-tile, BOTH heads of pair hp in one 2-bank psum
        # tile (h0 in cols 0:512, h1 in 512:1024). A single exp covers the
        # pair; the two K=64 matmuls run concurrently in disjoint PE halves.
        NG = MT

        def s_group(step):
            rnd, jt = step // NG, step % NG
            hp, ic = rnd // 4, rnd % 4
            ss = s_pool.tile([128, 1024], f32, tag="s",
                             name=f"s{hp}_{ic}_{jt}")
            for hh in range(2):
                po = hh * 64
                nc.tensor.matmul(
                    ss[:, hh * 512:(hh + 1) * 512],
                    lhsT=qkT[2 + hp][jt // 4][
                        po:po + 64, (jt % 4) * 128:(jt % 4 + 1) * 128],
                    rhs=qkT[hp][ic][po:po + 64, :],
                    start=True, stop=True)
            return ss

        def pv_normalize(hp, ic, pvs):
            # hh=1 first: its out_sb write goes through an SBUF->SBUF DMA,
            # which then overlaps hh=0's pure-DVE chain
            for hh in (1, 0):
                # release the pv psum slot quickly with a single copy, then
                # run the whole normalize chain from SBUF off-critical-path.
                # DVE ops stay lane-aligned; cross-partition moves use DMA.
                pv = pvs[hh]
                oa = z_pool.tile([128, 512], f32, tag="oa")
                nc.vector.tensor_copy(oa[:], pv[:])
                zd = zd_pool.tile([1, 512], f32, tag="zd")
                nc.sync.dma_start(zd[:], oa[64:65, :])
                zbz = z_pool.tile([64, 512], f32, tag="zbz")
                nc.sync.dma_start(
                    zbz[:], zd[0:1, :].to_broadcast([64, 512]))
                zb = z_pool.tile([64, 512], f32, tag="zb")
                nc.vector.reciprocal_approx_fast(zb[:], zbz[:])
                if hh == 0:
                    dst = out_sb[0:64, hp, ic * 512:(ic + 1) * 512]
                else:
                    dst = z_pool.tile([64, 512], bf16, tag="o1")
                nc.vector.tensor_mul(dst, oa[0:64, :], zb[:])
                if v_bias_nonzero:
                    h = 2 * hp + hh
                    nc.vector.tensor_scalar(
                        out=dst, in0=dst, scalar1=bv[0:64, h:h + 1],
                        scalar2=None, op0=add)
                if hh == 1:
                    nc.sync.dma_start(
                        out_sb[64:128, hp, ic * 512:(ic + 1) * 512],
                        dst[:])

        def proj_unit(it, oc, tail=False):
            ps = mm_pool.tile([128, 512], f32, tag="mm", name=f"y{it}_{oc}")
            for ct in range(2):
                nc.tensor.matmul(
                    ps[:],
                    lhsT=out_sb[:, ct, it * 128:(it + 1) * 128],
                    rhs=wp[:, ct, oc * 512:(oc + 1) * 512],
                    start=(ct == 0), stop=(ct == 1))
            ysb = y_pool.tile([128, 512], bf16, tag="y")
            # in the tail ACT is past the exp stream: split the psum drains
            # across ACT and DVE so they don't serialize on one engine
            if tail and oc == 1:
                nc.scalar.copy(ysb[:], ps[:])
            else:
                nc.vector.tensor_copy(ysb[:], ps[:])
            # alternate output queues (gpsimd's DGE dispatch is cheap and
            # idle after the input phase) so the final drain isn't serialized
            # on one queue; in the tail ACT is done with exps, so its queue
            # joins the rotation
            if tail:
                q = (nc.sync, nc.gpsimd, nc.scalar)[(2 * it + oc) % 3]
            else:
                q = nc.gpsimd if oc == 1 else nc.sync
            q.dma_start(
                y_d[it * 128:(it + 1) * 128, oc * 512:(oc + 1) * 512],
                ysb[:])

        def proj_block(ic, tail=False):
            for it in range(ic * 4, (ic + 1) * 4):
                for oc in range(2):
                    proj_unit(it, oc, tail=tail)

        def emit_filler(units):
            with tc.high_priority(offset=-20000):
                for u in units:
                    if u[0] == "qk":
                        qk_block(u[1], u[2])
                    elif u[0] == "v":
                        v_block(u[1])
                    else:
                        proj_block(u[1])

        # pre-loop: the immediately-needed k/q chunks and first v tiles at
        # normal priority, then ALL remaining qkv/v work in a low-priority
        # band ordered by consumption step. Everything is emitted before the
        # main loop (freshly-written tiles consumed as matmul lhsT raced on
        # hardware when produced just-in-time mid-loop), but the priority
        # band lets the scheduler slide it into PE gaps under the exp pace.
        # Round order is hp-outer (rnd = hp*4 + ic): k/q for head pair 1 are
        # only needed from step 64, and proj(ic) unlocks after round 4+ic.
        # All qkv/v production is emitted before the main loop (fresh tiles
        # consumed as matmul lhsT raced on hardware when produced mid-loop)
        # at natural priority, ordered by consumption time so the list
        # scheduler streams it into PE gaps in the right order.
        # k2 chunks first: the exp stream consumes them at steps 4/8/12 and
        # the mm-pool rotation would otherwise chain them behind the v
        # pipeline. PV can lag the exp stream by many steps (es_pool
        # elasticity), so v tiles follow.
        qk_block(2, 0)
        qk_block(0, 0)
        for mc in range(1, 4):
            qk_block(2, mc)
        qk_block(0, 1)          # hard deadline: exp(16) (PV lag is elastic,
        for mt in range(8):     # the exp stream's score deps are not)
            v_block(mt)
        qk_block(0, 2)
        for mt in range(8, 16):
            v_block(mt)
        qk_block(0, 3)
        for args in ((3, 0), (1, 0), (3, 1), (3, 2), (3, 3),
                     (1, 1), (1, 2), (1, 3)):
            qk_block(*args)

        NSTEP = 8 * NG
        LOOK = 2
        with tc.high_priority():
            ss_q = {i: s_group(i) for i in range(LOOK)}
            pvs = None
            for st in range(NSTEP):
                rnd, jt = st // NG, st % NG
                hp, ic = rnd // 4, rnd % 4
                if jt == 0:
                    pvs = [pv_pool.tile([128, 512], f32, tag="pv",
                                        name=f"pv{hp}_{ic}_{i}")
                           for i in range(2)]
                es = es_pool.tile([128, 1024], bf16, tag="es")
                nc.scalar.activation(es[:], ss_q[st % LOOK][:], Exp)
                if st + LOOK < NSTEP:
                    ss_q[st % LOOK] = s_group(st + LOOK)
                for hh in range(2):
                    h = 2 * hp + hh
                    nc.tensor.matmul(
                        pvs[hh][0:65, :],
                        lhsT=v_sb[:, jt, h * 65:(h + 1) * 65],
                        rhs=es[:, hh * 512:(hh + 1) * 512],
                        start=(jt == 0), stop=(jt == MT - 1))
                if jt == NG - 1:
                    pv_normalize(hp, ic, pvs)
                    if rnd >= 4:
                        # above the qk fillers, below the exp pipeline: proj
                        # is the preferred PE gap filler once it unlocks
                        with tc.high_priority(offset=-10000):
                            proj_block(rnd - 4)
        # tail: only the last i-chunk's projection remains; top priority so
        # the scheduler drains it ahead of any leftovers
        with tc.high_priority():
            proj_block(3, tail=True)

    nc.compile()
    return nc


def _prep_inputs(x, w_qkv, b_qkv, w_proj):
    """Build the 8 per-core input maps (host-side shard + transpose + cast).
    The softmax SCALE is folded into the q weights/bias so the device exp
    needs no scale argument."""
    w3 = w_qkv.reshape(C, 3, H, D)
    b3 = b_qkv.reshape(3, H, D)
    in_maps = []
    for c in range(N_CORES):
        b, g = divmod(c, 4)
        hs = slice(g * HL, (g + 1) * HL)
        wq = w3[:, 0, hs, :].reshape(C, 256) * SCALE
        wk = w3[:, 1, hs, :].reshape(C, 256)
        wv = w3[:, 2, hs, :].reshape(C, 256)
        bq = b3[0, hs, :].reshape(256) * SCALE
        bk = b3[1, hs, :].reshape(256)
        bv = b3[2, hs, :].reshape(256)
        # q/k transposed layout: head pair (2j, 2j+1) shares an SBUF tile
        # with partition offsets 0/64 -> natural [256,1] order is fine:
        # tile t covers dims [t*128,(t+1)*128) = heads 2t,2t+1.
        in_maps.append({
            "xT": np.ascontiguousarray(x[b].T).astype(BF),
            "wqk": np.concatenate([wq, wk], axis=1).astype(BF),
            "wv": wv.astype(BF),
            "wp": w_proj[g * 256:(g + 1) * 256, :].astype(BF),
            "bqk": np.concatenate([bq, bk]).reshape(512, 1)
                     .astype(np.float32),
            "bv": np.ascontiguousarray(bv.reshape(4, 64).T)
                    .astype(np.float32),
        })
    return in_maps


def _get_program(v_bias_nonzero: bool):
    key = ("prog", v_bias_nonzero)
    if key not in _cache:
        _install_ntff_hook()
        _cache[key] = _build_program(v_bias_nonzero)
    return _cache[key]


def run(x, w_qkv, b_qkv, w_proj, b_proj, trace=False, trace_kwargs=None):
    from concourse import bass_utils
    bass_utils.upload_artifacts = lambda tmpdir: tmpdir  # no cloud upload

    x = np.asarray(x, dtype=np.float32)
    w_qkv = np.asarray(w_qkv, dtype=np.float32)
    b_qkv = np.asarray(b_qkv, dtype=np.float32)
    w_proj = np.asarray(w_proj, dtype=np.float32)
    b_proj = np.asarray(b_proj, dtype=np.float32)

    v_bias_nonzero = bool(np.any(b_qkv.reshape(3, H, D)[2] != 0.0))
    nc = _get_program(v_bias_nonzero)
    in_maps = _prep_inputs(x, w_qkv, b_qkv, w_proj)
    res = bass_utils.run_bass_kernel_spmd(
        nc, in_maps, list(range(N_CORES)), trace=trace,
        **(trace_kwargs or {}))

    out = np.zeros((B, N, C), dtype=np.float32)
    for b in range(B):
        acc = np.zeros((N, C), dtype=np.float32)
        for g in range(4):
            acc += res.results[b * 4 + g]["y"].astype(np.float32)
        out[b] = acc + b_proj
    return out, res


def kernel(x, w_qkv, b_qkv, w_proj, b_proj):
    out, _ = run(x, w_qkv, b_qkv, w_proj, b_proj, trace=False)
    return out
